# revision 1
# baseline (speedup 1.0000x reference)
"""Trainium2 Bass kernel for nn_ContextQueryAttention (B=64, H=128, C=1024, Q=128).

Sharding: pure data-parallel over batch — 8 batches per NeuronCore, SPMD on 8
cores. Params (tiny H-vectors) replicated to every core.

Math (masks are all-ones, so masked softmax == plain softmax; softmax shift
invariance lets each score layout carry only its per-partition-friendly bias):
  S = s0[c] + s1[q] + s2[c,q] + bias,  s2 = (c*cqw)^T q  (contraction over H)
  a_att = softmax_q(S): independent of s0/bias;  computed from ET = exp(s2^T + s1)
  b_att = softmax_c(S): independent of s1/bias;  computed from Ec = exp(s2 + s0)
  a^T = q^T @ A_T,     A_T = ET / colsum(ET)                 [H,C]
  tmp = Ec^T @ c^T,    tmp2 = tmp / db,  db = colsum_c(Ec)   [Q,H]
  b^T = tmp2^T @ A_T                                          [H,C]
  out[b] = rows [c; a^T; c*a^T; c*b^T]                        [4H, C]

Matmuls run in bf16 (fp32 PSUM accumulation); exp/normalizers in fp32.
"""

import numpy as np
from contextlib import ExitStack

import concourse.bass as bass
import concourse.bacc as bacc
import concourse.tile as tile
from concourse import mybir
from concourse.bass_utils import run_bass_kernel_spmd
from concourse.masks import make_identity

F32 = mybir.dt.float32
BF16 = mybir.dt.bfloat16
EXP = mybir.ActivationFunctionType.Exp
COPY = mybir.ActivationFunctionType.Copy

B, H, C, Q = 64, 128, 1024, 128
NCORES = 8
NB = B // NCORES  # batches per core
NCK = C // 128    # 8 column chunks of C


def _body(ctx: ExitStack, tc: tile.TileContext, c_in, q_in, ctxw_in, qw_in,
          cqw_in, out, nb: int):
    nc = tc.nc

    const = ctx.enter_context(tc.tile_pool(name="const", bufs=1))
    big = ctx.enter_context(tc.tile_pool(name="big", bufs=4))
    poolc = ctx.enter_context(tc.tile_pool(name="poolc", bufs=8))
    poolo = ctx.enter_context(tc.tile_pool(name="poolo", bufs=4))
    med = ctx.enter_context(tc.tile_pool(name="med", bufs=4))
    small = ctx.enter_context(tc.tile_pool(name="small", bufs=4))
    # PSUM budget (8 banks): psA 4 (shared 2KB slots) + psCT 2 + psMisc 2
    psA = ctx.enter_context(tc.tile_pool(name="psA", bufs=4, space="PSUM"))
    psCT = ctx.enter_context(tc.tile_pool(name="psCT", bufs=2, space="PSUM"))
    psMisc = ctx.enter_context(tc.tile_pool(name="psM", bufs=2, space="PSUM"))

    # --- per-core constants ---
    ident_f = const.tile([128, 128], F32)
    make_identity(nc, ident_f)
    ident_b = const.tile([128, 128], BF16)
    make_identity(nc, ident_b)
    ones_b = const.tile([128, 128], BF16)
    nc.vector.memset(ones_b, 1.0)
    ctxw = const.tile([128, 1], F32)
    nc.gpsimd.dma_start(ctxw, ctxw_in[:, :])
    qw = const.tile([128, 1], F32)
    nc.gpsimd.dma_start(qw, qw_in[:, :])
    cqw = const.tile([128, 1], F32)
    nc.gpsimd.dma_start(cqw, cqw_in[:, :])
    rcqw = const.tile([128, 1], F32)
    nc.vector.reciprocal(rcqw, cqw)

    for b in range(nb):
        # ---- loads; the c row-block of the output is written back as soon
        # as it lands so the out-DMA stream starts early ----
        c_sb = poolc.tile([128, C], F32, tag="c_sb")
        nc.sync.dma_start(c_sb, c_in[b])
        q_sb = med.tile([128, Q], F32, tag="q_sb")
        nc.sync.dma_start(q_sb, q_in[b])
        nc.sync.dma_start(out[b, 0:128, :], c_sb)
        # out3 holds the computed row-blocks [aT; c*aT; c*bT]
        out3 = poolo.tile([128, 3, C], F32, tag="out3")

        # ---- casts / scaled copies ----
        c_scaled = big.tile([128, C], BF16, tag="c_scaled")   # (c * cqw) in bf16
        nc.vector.tensor_scalar_mul(c_scaled, c_sb, cqw)
        q_bf = med.tile([128, Q], BF16, tag="q_bf")
        nc.vector.tensor_copy(q_bf, q_sb)

        # ---- misc PSUM scratch (single bank) ----
        misc = psMisc.tile([128, 260], F32, tag="misc")
        s1_ps = misc[:, 0:1]
        s0_ps = misc[:, 1:9]
        tmpdb_ps = misc[:, 128:257]   # tmp in [:,0:128], db in [:,128]
        tmp_ps = tmpdb_ps[:, 0:128]
        db_ps = tmpdb_ps[:, 128:129]

        # ---- s1[q] = sum_h q[h,q]*qw[h] (fp32, N=1) ----
        nc.tensor.matmul(s1_ps, q_sb, qw)
        s1_sb = small.tile([128, 1], F32, tag="s1")
        nc.vector.tensor_copy(s1_sb, s1_ps)

        # ---- qT via PE transpose (fp32), evac-cast to bf16 ----
        qT_ps = psA.tile([128, 128], F32, tag="psA")
        nc.tensor.transpose(qT_ps, q_sb, ident_f)
        qT_bf = small.tile([128, 128], BF16, tag="qT")
        nc.vector.tensor_copy(qT_bf, qT_ps)

        # ---- S_T halves + ET = exp(S_T + s1) ----
        ET = big.tile([128, C], BF16, tag="ET")
        for h2 in range(2):
            sl = slice(512 * h2, 512 * (h2 + 1))
            st = psA.tile([128, 512], F32, tag="psA")
            nc.tensor.matmul(st, q_bf, c_scaled[:, sl])
            nc.scalar.activation(ET[:, sl], st, EXP, bias=s1_sb, scale=1.0)

        # ---- s0 chunks + S chunks; Ec = exp(S_c) (es0 applied via cT) ----
        Ec = big.tile([128, NCK, 128], BF16, tag="Ec")
        for half in range(2):
            sc = psA.tile([128, 4, 128], F32, tag="psA")
            for j4 in range(4):
                j = half * 4 + j4
                csl = slice(128 * j, 128 * (j + 1))
                nc.tensor.matmul(s0_ps[:, j:j + 1], c_sb[:, csl], ctxw)
                nc.tensor.matmul(sc[:, j4, :], c_scaled[:, csl], q_bf)
            nc.scalar.activation(Ec[:, 4 * half:4 * half + 4, :], sc, EXP)

        # cT carries es0[c] (exp of s0, per-partition) and the cqw[h] scale
        # from c_scaled; col 128 holds es0 itself so the tmp matmul also
        # accumulates db = sum_c Ec*es0 in its last output column.
        cT = big.tile([128, NCK, 129], BF16, tag="cT")
        es0 = small.tile([128, 8], F32, tag="es0")
        nc.scalar.activation(es0, s0_ps, EXP)
        nc.scalar.activation(cT[:, :, 128:129], es0, COPY)
        for half in range(2):
            ct_ps = psCT.tile([128, 4, 128], BF16, tag="ct")
            for j4 in range(4):
                j = half * 4 + j4
                nc.tensor.transpose(ct_ps[:, j4, :],
                                    c_scaled[:, 128 * j:128 * (j + 1)], ident_b)
            for j4 in range(4):
                j = half * 4 + j4
                nc.vector.tensor_scalar_mul(cT[:, j, 0:128], ct_ps[:, j4, :],
                                            es0[:, j:j + 1])

        # ---- D_A = colsum(ET) broadcast; recD = 1/D_A; A_T = ET*recD ----
        recD = big.tile([128, C], F32, tag="recD")
        for h2 in range(2):
            sl = slice(512 * h2, 512 * (h2 + 1))
            da = psA.tile([128, 512], F32, tag="psA")
            nc.tensor.matmul(da, ones_b, ET[:, sl])
            nc.vector.reciprocal(recD[:, sl], da)
        A_T = big.tile([128, C], BF16, tag="A_T")
        nc.vector.tensor_mul(A_T[:, 0:512], ET[:, 0:512], recD[:, 0:512])
        nc.gpsimd.tensor_mul(A_T[:, 512:], ET[:, 512:], recD[:, 512:])

        # ---- [tmp | db] = sum_j Ec_j^T @ [cs0T_j | es0_j] (fused, one group) ----
        for j in range(NCK):
            nc.tensor.matmul(tmpdb_ps, Ec[:, j, :], cT[:, j, :],
                             start=(j == 0), stop=(j == NCK - 1))
        rdb = small.tile([128, 1], F32, tag="rdb")
        nc.vector.reciprocal(rdb, db_ps)
        tmp2 = small.tile([128, 128], BF16, tag="tmp2")
        nc.vector.tensor_scalar_mul(tmp2, tmp_ps, rdb)

        # ---- aT = qT^T @ A_T ; bT = (1/cqw) * (tmp2^T @ A_T) (halves) ----
        aT_sb = out3[:, 0, :]
        bT_sb = big.tile([128, C], F32, tag="bT_sb")
        for h2 in range(2):
            sl = slice(512 * h2, 512 * (h2 + 1))
            ap = psA.tile([128, 512], F32, tag="psA")
            nc.tensor.matmul(ap, qT_bf, A_T[:, sl])
            nc.scalar.activation(aT_sb[:, sl], ap, COPY)
        for h2 in range(2):
            sl = slice(512 * h2, 512 * (h2 + 1))
            bp = psA.tile([128, 512], F32, tag="psA")
            nc.tensor.matmul(bp, tmp2, A_T[:, sl])
            nc.scalar.activation(bT_sb[:, sl], bp, COPY, scale=rcqw)

        # ---- elementwise products (ca on Pool ∥ cb on DVE; cb is the
        # later product, so it gets the faster engine) ----
        nc.gpsimd.tensor_mul(out3[:, 1, :], c_sb, aT_sb)
        nc.vector.tensor_mul(out3[:, 2, :], c_sb, bT_sb)

        # ---- store: one DMA for the 3 computed row-blocks ----
        nc.sync.dma_start(
            out[b, 128:512, :].rearrange("(k h) c -> h k c", h=128), out3)


def build_nc(nb: int = NB) -> bass.Bass:
    nc = bacc.Bacc("TRN2", target_bir_lowering=False, debug=False)
    c_in = nc.declare_dram_parameter("c", [nb, H, C], F32, isOutput=False)
    q_in = nc.declare_dram_parameter("q", [nb, H, Q], F32, isOutput=False)
    ctxw = nc.declare_dram_parameter("ctxw", [H, 1], F32, isOutput=False)
    qw = nc.declare_dram_parameter("qw", [H, 1], F32, isOutput=False)
    cqw = nc.declare_dram_parameter("cqw", [H, 1], F32, isOutput=False)
    out = nc.declare_dram_parameter("out", [nb, 4 * H, C], F32, isOutput=True)
    with tile.TileContext(nc) as tc:
        with ExitStack() as ctx:
            _body(ctx, tc, c_in[:], q_in[:], ctxw[:], qw[:], cqw[:], out[:], nb)
    nc.compile()
    return nc


_NC_CACHE: dict = {}


def _get_nc(nb: int) -> bass.Bass:
    if nb not in _NC_CACHE:
        _NC_CACHE[nb] = build_nc(nb)
    return _NC_CACHE[nb]


def make_in_maps(inputs: dict, ncores: int = NCORES):
    c = np.ascontiguousarray(np.asarray(inputs["c"], dtype=np.float32))
    q = np.ascontiguousarray(np.asarray(inputs["q"], dtype=np.float32))
    ctxw = np.ascontiguousarray(
        np.asarray(inputs["context_weights"], np.float32).reshape(H, 1))
    qw = np.ascontiguousarray(
        np.asarray(inputs["query_weights"], np.float32).reshape(H, 1))
    cqw = np.ascontiguousarray(
        np.asarray(inputs["cq_weights"], np.float32).reshape(H, 1))
    nb = c.shape[0] // ncores
    return [
        {
            "c": c[i * nb:(i + 1) * nb],
            "q": q[i * nb:(i + 1) * nb],
            "ctxw": ctxw,
            "qw": qw,
            "cqw": cqw,
        }
        for i in range(ncores)
    ], nb


def kernel(**inputs) -> np.ndarray:
    in_maps, nb = make_in_maps(inputs)
    nc = _get_nc(nb)
    res = run_bass_kernel_spmd(nc, in_maps, list(range(NCORES)))
    return np.concatenate([res.results[i]["out"] for i in range(NCORES)], axis=0)



# revision 15
# speedup vs baseline: 1.1956x; 1.1956x over previous
"""Trainium2 Bass kernel for nn_ContextQueryAttention (B=64, H=128, C=1024, Q=128).

Sharding: pure data-parallel over batch — 8 batches per NeuronCore, SPMD on 8
cores. Params (tiny H-vectors) replicated to every core.

Math (masks are all-ones, so masked softmax == plain softmax; softmax shift
invariance lets each score layout carry only its per-partition-friendly bias):
  S = s0[c] + s1[q] + s2[c,q] + bias,  s2 = (c*cqw)^T q  (contraction over H)
  a_att = softmax_q(S): independent of s0/bias;  computed from ET = exp(s2^T + s1)
  b_att = softmax_c(S): independent of s1/bias;  computed from Ec = exp(s2 + s0)
  a^T = q^T @ A_T,     A_T = ET / colsum(ET)                 [H,C]
  tmp = Ec^T @ c^T,    tmp2 = tmp / db,  db = colsum_c(Ec)   [Q,H]
  b^T = tmp2^T @ A_T                                          [H,C]
  out[b] = rows [c; a^T; c*a^T; c*b^T]                        [4H, C]

DMA schedule: the kernel is HBM-bandwidth bound (20.5 MB/core through a
serialized DMA path), so every input load is issued up front on the SP queue
(dedicated SBUF buffers per batch, no recycle waits) with the c row-block
bounced straight back out, and each computed row-block is stored with its own
DMA as soon as it is produced. This keeps the DMA engines continuously busy
instead of stalling behind a monolithic end-of-batch store.

Matmuls run in bf16 (fp32 PSUM accumulation); exp/normalizers in fp32.
"""

import numpy as np
from contextlib import ExitStack

import concourse.bass as bass
import concourse.bacc as bacc
import concourse.tile as tile
from concourse import mybir
from concourse.bass_utils import run_bass_kernel_spmd
from concourse.masks import make_identity

F32 = mybir.dt.float32
BF16 = mybir.dt.bfloat16
EXP = mybir.ActivationFunctionType.Exp
COPY = mybir.ActivationFunctionType.Copy

B, H, C, Q = 64, 128, 1024, 128
NCORES = 8
NB = B // NCORES  # batches per core
NCK = C // 128    # 8 column chunks of C


def _body(ctx: ExitStack, tc: tile.TileContext, c_in, q_in, ctxw_in, qw_in,
          cqw_in, out, nb: int):
    nc = tc.nc

    const = ctx.enter_context(tc.tile_pool(name="const", bufs=1))
    poolc = ctx.enter_context(tc.tile_pool(name="poolc", bufs=1))
    poolq = ctx.enter_context(tc.tile_pool(name="poolq", bufs=1))
    big = ctx.enter_context(tc.tile_pool(name="big", bufs=4))
    poolo = ctx.enter_context(tc.tile_pool(name="poolo", bufs=8))
    med = ctx.enter_context(tc.tile_pool(name="med", bufs=4))
    small = ctx.enter_context(tc.tile_pool(name="small", bufs=4))
    # PSUM budget (8 banks): psA 4 (shared 2KB slots) + psCT 2 + psMisc 2
    psA = ctx.enter_context(tc.tile_pool(name="psA", bufs=4, space="PSUM"))
    psCT = ctx.enter_context(tc.tile_pool(name="psCT", bufs=2, space="PSUM"))
    psMisc = ctx.enter_context(tc.tile_pool(name="psM", bufs=2, space="PSUM"))

    # --- per-core constants (tiny DMAs on the Pool queue: they slot into
    # device gaps without occupying SP dispatch slots) ---
    ctxw = const.tile([128, 1], F32)
    nc.gpsimd.dma_start(ctxw, ctxw_in[:, :])
    qw = const.tile([128, 1], F32)
    nc.gpsimd.dma_start(qw, qw_in[:, :])
    cqw = const.tile([128, 1], F32)
    nc.gpsimd.dma_start(cqw, cqw_in[:, :])

    # --- input load train: all batches up front ---
    cs, qs = [], []
    for b in range(nb):
        c_sb = poolc.tile([128, C], F32, tag=f"c{b}", name=f"c_sb{b}")
        nc.sync.dma_start(c_sb, c_in[b])
        q_sb = poolq.tile([128, Q], F32, tag=f"q{b}", name=f"q_sb{b}")
        nc.sync.dma_start(q_sb, q_in[b])
        cs.append(c_sb)
        qs.append(q_sb)
    # c row-block of the output: HBM->HBM copy, no SBUF dependency (the
    # same bytes move over the bus either way; this frees it from the
    # 900ns load-completion sem propagation)
    for b in range(nb):
        nc.sync.dma_start(out[b, 0:128, :], c_in[b])

    ident_f = const.tile([128, 128], F32)
    make_identity(nc, ident_f)
    ident_b = const.tile([128, 128], BF16)
    make_identity(nc, ident_b)
    ones_b = const.tile([128, 128], BF16)
    nc.vector.memset(ones_b, 1.0)

    for b in range(nb):
        c_sb = cs[b]
        q_sb = qs[b]

        # ---- casts: q absorbs the cqw scale (so b^T needs no 1/cqw fixup);
        # the plain bf16 c cast is split across Act and DVE ----
        q_cq = med.tile([128, Q], BF16, tag="q_cq")
        nc.vector.tensor_scalar_mul(q_cq, q_sb, cqw)
        c_bf = big.tile([128, C], BF16, tag="c_bf")
        nc.scalar.activation(c_bf[:, 0:512], c_sb[:, 0:512], COPY)
        nc.gpsimd.tensor_copy(c_bf[:, 512:], c_sb[:, 512:])

        # ---- misc PSUM scratch (single bank) ----
        misc = psMisc.tile([128, 260], F32, tag="misc")
        s1_ps = misc[:, 0:1]
        s0_ps = misc[:, 1:9]
        tmpdb_ps = misc[:, 128:257]   # tmp in [:,0:128], db in [:,128]
        tmp_ps = tmpdb_ps[:, 0:128]
        db_ps = tmpdb_ps[:, 128:129]

        # ---- s1[q] = sum_h q[h,q]*qw[h] (fp32, N=1) ----
        nc.tensor.matmul(s1_ps, q_sb, qw)
        s1_sb = small.tile([128, 1], F32, tag="s1")
        nc.vector.tensor_copy(s1_sb, s1_ps)

        # ---- qT via PE transpose of raw fp32 q, evac casts to bf16 ----
        qT_ps = psA.tile([128, 128], F32, tag="psA")
        nc.tensor.transpose(qT_ps, q_sb, ident_f)
        qT_bf = small.tile([128, 128], BF16, tag="qT")
        nc.vector.tensor_copy(qT_bf, qT_ps)

        # ---- S_T halves + ET = exp(S_T + s1) ----
        ET = big.tile([128, C], BF16, tag="ET")
        for h2 in range(2):
            sl = slice(512 * h2, 512 * (h2 + 1))
            st = psA.tile([128, 512], F32, tag="psA")
            nc.tensor.matmul(st, q_cq, c_bf[:, sl])
            nc.scalar.activation(ET[:, sl], st, EXP, bias=s1_sb, scale=1.0)

        # ---- s0 chunks + S chunks; Ec = exp(S_c) (es0 applied via cT) ----
        Ec = big.tile([128, NCK, 128], BF16, tag="Ec")
        for half in range(2):
            sc = psA.tile([128, 4, 128], F32, tag="psA")
            for j4 in range(4):
                j = half * 4 + j4
                csl = slice(128 * j, 128 * (j + 1))
                nc.tensor.matmul(s0_ps[:, j:j + 1], c_sb[:, csl], ctxw)
                nc.tensor.matmul(sc[:, j4, :], c_bf[:, csl], q_cq)
            nc.scalar.activation(Ec[:, 4 * half:4 * half + 4, :], sc, EXP)

        # cT carries es0[c] (exp of s0, per-partition); col 128 holds es0
        # itself so the tmp matmul also accumulates db = sum_c Ec*es0 in its
        # last output column.
        cT = big.tile([128, NCK, 129], BF16, tag="cT")
        es0 = small.tile([128, 8], F32, tag="es0")
        nc.scalar.activation(es0, s0_ps, EXP)
        nc.scalar.activation(cT[:, :, 128:129], es0, COPY)
        for half in range(2):
            ct_ps = psCT.tile([128, 4, 128], BF16, tag="ct")
            for j4 in range(4):
                j = half * 4 + j4
                nc.tensor.transpose(ct_ps[:, j4, :],
                                    c_bf[:, 128 * j:128 * (j + 1)], ident_b)
            for j4 in range(4):
                j = half * 4 + j4
                nc.vector.tensor_scalar_mul(cT[:, j, 0:128], ct_ps[:, j4, :],
                                            es0[:, j:j + 1])

        # ---- D_A = colsum(ET) broadcast (bf16); recD = 1/D_A; A_T = ET*recD
        # (all-bf16 so the DVE ops hit the fast perf modes) ----
        recD = big.tile([128, C], BF16, tag="recD")
        for h2 in range(2):
            sl = slice(512 * h2, 512 * (h2 + 1))
            da = psA.tile([128, 512], F32, tag="psA")
            nc.tensor.matmul(da, ones_b, ET[:, sl])
            with nc.allow_low_precision(reason="1/D in bf16: 0.4% rel, tol 2e-2"):
                nc.vector.reciprocal(recD[:, sl], da)
        A_T = big.tile([128, C], BF16, tag="A_T")
        nc.vector.tensor_mul(A_T[:, 0:512], ET[:, 0:512], recD[:, 0:512])
        nc.vector.tensor_mul(A_T[:, 512:], ET[:, 512:], recD[:, 512:])

        # ---- [tmp | db] = sum_j Ec_j^T @ [cs0T_j | es0_j] (fused, one group) ----
        for j in range(NCK):
            nc.tensor.matmul(tmpdb_ps, Ec[:, j, :], cT[:, j, :],
                             start=(j == 0), stop=(j == NCK - 1))
        rdb = small.tile([128, 1], F32, tag="rdb")
        nc.vector.reciprocal(rdb, db_ps)
        tmp2 = small.tile([128, 128], BF16, tag="tmp2")
        nc.vector.tensor_scalar_mul(tmp2, tmp_ps, rdb)

        # ---- aT = qT^T @ A_T (Act evacs); bT = tmp2^T @ A_T stays in PSUM
        # and feeds the cb product directly ----
        aT_sb = poolo.tile([128, C], F32, tag="aT")
        for h2 in range(2):
            sl = slice(512 * h2, 512 * (h2 + 1))
            ap = psA.tile([128, 512], F32, tag="psA")
            nc.tensor.matmul(ap, qT_bf, A_T[:, sl])
            nc.scalar.activation(aT_sb[:, sl], ap, COPY)
        nc.sync.dma_start(out[b, 128:256, :], aT_sb)

        # ---- elementwise products; each row-block stored as soon as ready ----
        ca_sb = poolo.tile([128, C], F32, tag="ca")
        nc.gpsimd.tensor_mul(ca_sb, c_sb, aT_sb)
        nc.sync.dma_start(out[b, 256:384, :], ca_sb)

        cb_sb = poolo.tile([128, C], F32, tag="cb")
        for h2 in range(2):
            sl = slice(512 * h2, 512 * (h2 + 1))
            bp = psA.tile([128, 512], F32, tag="psA")
            nc.tensor.matmul(bp, tmp2, A_T[:, sl])
            nc.vector.tensor_mul(cb_sb[:, sl], c_sb[:, sl], bp)
        nc.sync.dma_start(out[b, 384:512, :], cb_sb)


def build_nc(nb: int = NB) -> bass.Bass:
    nc = bacc.Bacc("TRN2", target_bir_lowering=False, debug=False)
    c_in = nc.declare_dram_parameter("c", [nb, H, C], F32, isOutput=False)
    q_in = nc.declare_dram_parameter("q", [nb, H, Q], F32, isOutput=False)
    ctxw = nc.declare_dram_parameter("ctxw", [H, 1], F32, isOutput=False)
    qw = nc.declare_dram_parameter("qw", [H, 1], F32, isOutput=False)
    cqw = nc.declare_dram_parameter("cqw", [H, 1], F32, isOutput=False)
    out = nc.declare_dram_parameter("out", [nb, 4 * H, C], F32, isOutput=True)
    with tile.TileContext(nc) as tc:
        with ExitStack() as ctx:
            _body(ctx, tc, c_in[:], q_in[:], ctxw[:], qw[:], cqw[:], out[:], nb)
    nc.compile()
    return nc


_NC_CACHE: dict = {}


def _get_nc(nb: int) -> bass.Bass:
    if nb not in _NC_CACHE:
        _NC_CACHE[nb] = build_nc(nb)
    return _NC_CACHE[nb]


def make_in_maps(inputs: dict, ncores: int = NCORES):
    c = np.ascontiguousarray(np.asarray(inputs["c"], dtype=np.float32))
    q = np.ascontiguousarray(np.asarray(inputs["q"], dtype=np.float32))
    ctxw = np.ascontiguousarray(
        np.asarray(inputs["context_weights"], np.float32).reshape(H, 1))
    qw = np.ascontiguousarray(
        np.asarray(inputs["query_weights"], np.float32).reshape(H, 1))
    cqw = np.ascontiguousarray(
        np.asarray(inputs["cq_weights"], np.float32).reshape(H, 1))
    nb = c.shape[0] // ncores
    return [
        {
            "c": c[i * nb:(i + 1) * nb],
            "q": q[i * nb:(i + 1) * nb],
            "ctxw": ctxw,
            "qw": qw,
            "cqw": cqw,
        }
        for i in range(ncores)
    ], nb


def kernel(**inputs) -> np.ndarray:
    in_maps, nb = make_in_maps(inputs)
    nc = _get_nc(nb)
    res = run_bass_kernel_spmd(nc, in_maps, list(range(NCORES)))
    return np.concatenate([res.results[i]["out"] for i in range(NCORES)], axis=0)
